# revision 20
# baseline (speedup 1.0000x reference)
"""Trainium2 Bass kernel for nn_MultiAgentsSummarizer (pointer-generator style
multi-agent summarizer distribution).

Math (per batch b, with T=64 target positions, A=4 agents, S=512 source tokens,
V=32000 vocab, EXT_V=33000 extended vocab):

    coef[t]   = sum_a agent_attn[t,a] * gen[t,a]
    out[t,v]  = coef[t] * vocab_probs[t,v]            (v <  V;  0 for v >= V)
    out[t, article[a,s]] += agent_attn[t,a]*(1-gen[t,a]) * agentwise_attn[t,a,s]

Strategy: one batch element per NeuronCore (B=8 = n_cores). Device work runs
v-major in 258 blocks of 128 rows (NB*128 = 33024 >= EXT_V). Both the base
term and the scatter term for a block are PE matmuls accumulating into the
same PSUM region:

  base    psum[p,t] += sum_k vocabT[k=t, blk*128+p] * coefdiag[k=t, t']
          (lhsT = vocab slice in natural [T, V] layout, rhs = diag(coef);
          vocab halves stacked on partitions so base partition is 0 or 64)
  scatter psum[p,t] += sum_k sel[k, p] * w[k, t]
          (host sorts the A*S=2048 contributions by destination block;
          32 slots per block, 3 blocks per 128-slot chunk at partition
          offsets {0,32,64}; sel is the 0/1 row-selector; duplicates need
          no special handling -- the matmul sums them)

w[k,t] = agentwise_attn[t,a(k),s(k)] * c4[t,a(k)] is built on device: a tiny
one-hot matmul gathers c4 rows per slot (c4sel), then one DVE multiply per
4-chunk span. PSUM group rule (one pending accumulation group per 2KB bank)
is satisfied by emitting base->scatter back-to-back per block. Groups of 6
blocks share one PSUM bank [128, 384]; ACT/DVE alternate copying PSUM to
bf16 tiles which stream out with per-partition-contiguous DMA. All heavy
tensors travel bf16 (tolerance 2e-2; bf16 error ~3e-3). The host only
reorders/relabels/casts -- all arithmetic runs on device.
"""

import numpy as np
import ml_dtypes

import concourse.bacc as bacc
import concourse.bass as bass
import concourse.mybir as mybir
import concourse.tile as tile
from concourse.bass_utils import run_bass_kernel_spmd

B, T, A, S = 8, 64, 4, 512
V, EXT_V = 32000, 33000
KC = A * S  # 2048 contributions per batch element
BF = ml_dtypes.bfloat16
F8 = ml_dtypes.float8_e4m3

NB = 258  # v-blocks of 128 rows; NB*128 = 33024 >= EXT_V
VB = 250  # blocks with vocab rows (V = 250*128 exactly)
SPB = 32  # payload slots per block (max contributions per block)
BPC = 3  # blocks per 128-slot chunk (partition offsets 0/32/64)
NCHUNK = NB // BPC  # 86
GB = 8  # blocks per psum group (one full 2KB PSUM bank [128, 512])
NG = (NB + GB - 1) // GB  # 33: 32 full groups + tail of 2 blocks
VHALF = V // 2  # 16000 columns per stacked vocab half

_prog = None


class _nullctx:
    def __enter__(self):
        return None

    def __exit__(self, *a):
        return False


def _build_program(loop_n=None, ablate=(), sel_f8=True):
    """loop_n: on-device repeat loop (bench variant; outputs then meaningless).
    ablate: subset of {"scatter", "base", "w", "copies"} (bench attribution)."""
    ablate = set(ablate)
    if "w" in ablate:
        ablate.add("scatter")
    nc = bacc.Bacc("TRN2", target_bir_lowering=False)
    f32 = mybir.dt.float32
    b16 = mybir.dt.bfloat16
    f8 = mybir.dt.float8e4 if sel_f8 else mybir.dt.bfloat16
    # bulk tensors travel as f32-typed DMA (bitcast): HW DMA throughput is
    # per-element, so 2-byte/1-byte elements would halve/quarter bandwidth
    SELW = (NCHUNK * 128) // (4 if sel_f8 else 2)
    vocab_in = nc.dram_tensor("vocab_in", [128, VHALF // 2], f32, kind="ExternalInput")
    attn_in = nc.dram_tensor("attn_in", [128, NCHUNK * T // 2], f32, kind="ExternalInput")
    sel_in = nc.dram_tensor("sel_in", [128, SELW], f32, kind="ExternalInput")
    oha_in = nc.dram_tensor("oha_in", [A, SELW], f32, kind="ExternalInput")
    gen_t = nc.dram_tensor("gen_t", [A, T], f32, kind="ExternalInput")
    agat_t = nc.dram_tensor("agat_t", [A, T], f32, kind="ExternalInput")
    mask2_in = nc.dram_tensor("mask2_in", [128, T], b16, kind="ExternalInput")
    out_img = nc.dram_tensor("out_img", [128, NB * T // 2], f32, kind="ExternalOutput")

    with tile.TileContext(nc) as tc:
        with (
            tc.tile_pool(name="small", bufs=1) as small,
            tc.tile_pool(name="big", bufs=1) as big,
            tc.tile_pool(name="outp", bufs=4) as outp,
            tc.tile_pool(name="psc", bufs=1, space="PSUM") as psc,
            tc.tile_pool(name="psg", bufs=2, space="PSUM") as psg,
            tc.tile_pool(name="psa", bufs=5, space="PSUM") as psa,
            (tc.For_i(0, loop_n, 1) if loop_n else _nullctx()),
        ):
            # ---- tiny loads (SP queue) ----
            gen_sb = small.tile([A, T], f32)
            nc.sync.dma_start(gen_sb[:], gen_t[:])
            agat_sb = small.tile([A, T], f32)
            nc.sync.dma_start(agat_sb[:], agat_t[:])
            mask2 = small.tile([128, T], b16)
            nc.sync.dma_start(mask2[:], mask2_in[:])
            # ---- big loads. ACT issues NO DMAs: a queue's dma transfer
            # blocks its later compute instructions, and ACT must start
            # copying PSUM early. Interleave on SP and Pool so the PE's
            # in-order stream (which alternates base/scatter per block)
            # never waits long: oha/attn gate the w-pipeline, sel gates the
            # first scatter, vocab chunks pace the base matmuls.
            asb = big.tile([128, NCHUNK * T], b16)
            selsb = big.tile([128, NCHUNK * 128], f8)
            ohasb = small.tile([A, NCHUNK * 128], f8)
            vsb = big.tile([128, VHALF], b16)
            ahw = NCHUNK * T // 2
            shw = NCHUNK * 128 // 2
            ohw = NCHUNK * 128 // 2
            NVD = 8
            vw = VHALF // NVD
            # Pool ring: oha1, sel1, oha2, then odd vocab chunks; sel2 rides
            # between late vocab chunks (needed only when PE passes block 129)
            ohf = ohasb[:].bitcast(f32)
            self_ = selsb[:].bitcast(f32)
            asf = asb[:].bitcast(f32)
            vsf = vsb[:].bitcast(f32)
            ohw4 = SELW // 2
            shw4 = SELW // 2
            ahw4 = NCHUNK * T // 4
            vw4 = vw // 2
            nc.gpsimd.dma_start(ohf[:, 0:ohw4], oha_in[:, 0:ohw4])
            nc.gpsimd.dma_start(self_[:, 0:shw4], sel_in[:, 0:shw4])
            nc.gpsimd.dma_start(ohf[:, ohw4:], oha_in[:, ohw4:])
            # SP ring: (smalls above), attn1, then even vocab chunks
            nc.sync.dma_start(asf[:, 0:ahw4], attn_in[:, 0:ahw4])
            for k in range(NVD):
                eng = nc.sync if k % 2 == 0 else nc.gpsimd
                eng.dma_start(
                    vsf[:, k * vw4 : (k + 1) * vw4], vocab_in[:, k * vw4 : (k + 1) * vw4]
                )
                if k == 3:
                    nc.sync.dma_start(asf[:, ahw4:], attn_in[:, ahw4:])
                if k == 4:
                    nc.gpsimd.dma_start(self_[:, shw4:], sel_in[:, shw4:])

            # ---- coefficients ----
            prod = small.tile([A, T], f32)
            nc.vector.tensor_mul(prod[:], agat_sb[:], gen_sb[:])
            c4t_f = small.tile([A, T], f32)  # agent_attn*(1-gen) = agat - prod
            nc.vector.tensor_sub(c4t_f[:], agat_sb[:], prod[:])
            c4t = small.tile([A, T], b16)
            nc.vector.tensor_copy(c4t[:], c4t_f[:])

            ones4 = small.tile([A, 128], f32)
            nc.vector.memset(ones4[:], 1.0)
            warm_ps = psc.tile([128, T], f32, space="PSUM", tag="warm")
            for _ in range(24):  # spin the PE p-state up during the load fill
                nc.tensor.matmul(warm_ps[:], lhsT=ones4[:], rhs=prod[:], start=True, stop=True)
            coef_ps = psc.tile([128, T], f32, space="PSUM")
            nc.tensor.matmul(coef_ps[:], lhsT=ones4[:], rhs=prod[:], start=True, stop=True)
            coef_bc = small.tile([128, T], b16)
            nc.scalar.copy(coef_bc[:], coef_ps[:])
            coefdiag = small.tile([128, T], b16)  # rows 0..63 & 64..127 = diag(coef)
            nc.vector.tensor_mul(coefdiag[:], mask2[:], coef_bc[:])
            zlhs = small.tile([64, 128], b16)
            nc.vector.memset(zlhs[:], 0.0)

            # ---- payload w[slot, t] = attn[slot, t] * c4[t, a(slot)] ----
            wsb = None
            if "w" not in ablate:
                wsb = big.tile([128, NCHUNK * T], b16)
            if "w" not in ablate:
                CPG = 8  # chunks per c4sel psum tile
                for cg in range((NCHUNK + CPG - 1) // CPG):
                    c0 = cg * CPG
                    c1 = min(c0 + CPG, NCHUNK)
                    ncr = c1 - c0
                    c4s = psg.tile([128, CPG * T], f32, space="PSUM", tag="c4s")
                    for j in range(ncr):
                        nc.tensor.matmul(
                            c4s[:, j * T : (j + 1) * T],
                            lhsT=ohasb[:, (c0 + j) * 128 : (c0 + j + 1) * 128],
                            rhs=c4t[:],
                            start=True,
                            stop=True,
                        )
                    nc.vector.tensor_mul(
                        wsb[:, c0 * T : c1 * T],
                        asb[:, c0 * T : c1 * T],
                        c4s[:, 0 : ncr * T],
                    )

            # ---- main loop: 43 groups of 6 blocks ----
            do_base = "base" not in ablate
            do_scat = "scatter" not in ablate
            osb = None
            if "copies" not in ablate:
                osb = big.tile([128, NB * T], b16)
            WG = 2  # groups per wave: all bases emitted before the scatters,
            # so the in-order PE queue has base work queued while scatter
            # operands (sel/w) are still arriving. One wave = one vocab chunk.
            for w0 in range(0, NG, WG):
                wgs = range(w0, min(w0 + WG, NG))
                accs = {}
                for g in wgs:
                    gb = min(GB, NB - g * GB)
                    acc_t = psa.tile([128, GB * T], f32, space="PSUM", tag="acc")
                    accs[g] = (acc_t, gb)
                # one PSUM bank allows a single pending accumulation group:
                # emit base->scatter back-to-back per block
                for g in wgs:
                    acc, gb = accs[g]
                    for i in range(gb):
                        j = g * GB + i
                        reg = acc[:, i * T : (i + 1) * T]
                        if do_base:
                            if j < VB // 2:
                                lhs = vsb[0:64, j * 128 : (j + 1) * 128]
                                rhs = coefdiag[0:64, :]
                            elif j < VB:
                                lhs = vsb[64:128, (j - VB // 2) * 128 : (j - VB // 2 + 1) * 128]
                                rhs = coefdiag[64:128, :]
                            else:
                                # zero-start from tile position (0,0): HW rejects
                                # start=True matmuls at row-offset tile positions
                                lhs = zlhs[:]
                                rhs = coefdiag[0:64, :]
                            nc.tensor.matmul(reg, lhsT=lhs, rhs=rhs, start=True, stop=not do_scat)
                        if do_scat:
                            c = j // BPC
                            off = (j % BPC) * 32
                            nc.tensor.matmul(
                                reg,
                                lhsT=selsb[off : off + 32, c * 128 : (c + 1) * 128],
                                rhs=wsb[off : off + 32, c * T : (c + 1) * T],
                                start=not do_base,
                                stop=True,
                            )
                if "copies" in ablate:
                    continue
                for g in wgs:
                    acc, gb = accs[g]
                    oreg = osb[:, g * GB * T : (g * GB + gb) * T]
                    if g % 5 == 2 or g % 5 == 4:
                        nc.vector.tensor_copy(oreg, acc[:, 0 : gb * T])
                    else:
                        nc.scalar.copy(oreg, acc[:, 0 : gb * T])
                g = wgs[-1]
                gb = accs[g][1]
                col0, col1 = w0 * GB * T, (g * GB + gb) * T
                eng = (nc.gpsimd, nc.sync, nc.scalar)[(w0 // WG) % 3]
                eng.dma_start(out_img[:, col0:col1], osb[:, col0:col1])

    nc.compile()
    return nc


_MASK2 = (np.arange(128)[:, None] % 64 == np.arange(T)[None, :]).astype(BF)


def _pack_core(vocab_b, gen_b, agat_b, attn_b, article_b, sel_f8=True):
    """Host-side data layout for one batch element (reorder/relabel/cast only)."""
    vocab_img = np.ascontiguousarray(
        vocab_b.reshape(T, 2, VHALF).transpose(1, 0, 2).reshape(128, VHALF)
    ).astype(BF)

    v = article_b.reshape(-1).astype(np.int64)  # contribution k = a*S + s
    blk = v >> 7
    part = v & 127
    order = np.argsort(blk, kind="stable")
    blk_s = blk[order]
    part_s = part[order]
    counts = np.bincount(blk_s, minlength=NB)
    if counts.max() > SPB:
        raise RuntimeError(f"block capacity exceeded: {counts.max()} > {SPB}")
    starts = np.zeros(NB + 1, np.int64)
    np.cumsum(counts, out=starts[1:])
    rank = np.arange(KC) - starts[blk_s]
    srow = (blk_s // BPC) * 128 + (blk_s % BPC) * 32 + rank  # slot row

    attn_kt = attn_b.reshape(T, KC).T  # [k, t]
    Wf = np.zeros((NCHUNK * 128, T), np.float32)
    Wf[srow] = attn_kt[order]
    attn_img = np.ascontiguousarray(
        Wf.reshape(NCHUNK, 128, T).transpose(1, 0, 2).reshape(128, NCHUNK * T)
    ).astype(BF)

    Sf = np.zeros((NCHUNK * 128, 128), np.float32)
    Sf[srow, part_s] = 1.0
    sel_img = np.ascontiguousarray(
        Sf.reshape(NCHUNK, 128, 128).transpose(1, 0, 2).reshape(128, NCHUNK * 128)
    ).astype(F8 if sel_f8 else BF)

    a_of = np.arange(KC) // S
    Oh = np.zeros((A, NCHUNK * 128), np.float32)
    Oh[a_of[order], srow] = 1.0

    return {
        "vocab_in": vocab_img.view(np.float32),
        "attn_in": attn_img.view(np.float32),
        "sel_in": sel_img.view(np.float32),
        "oha_in": np.ascontiguousarray(Oh.astype(F8 if sel_f8 else BF)).view(np.float32),
        "gen_t": np.ascontiguousarray(gen_b.T, dtype=np.float32),
        "agat_t": np.ascontiguousarray(agat_b.T, dtype=np.float32),
        "mask2_in": _MASK2,
    }


def _unpack(res):
    """out_img [128, NB*T/2] f32 (bitcast bf16 pairs) -> [T, EXT_V] f32."""
    raw = np.ascontiguousarray(np.asarray(res["out_img"])).view(BF)
    x = raw.astype(np.float32).reshape(128, NB, T)
    return x.transpose(2, 1, 0).reshape(T, NB * 128)[:, :EXT_V]


def kernel(vocab_probs, generation_probs, agentwise_attn, agent_attn, article):
    global _prog
    vocab_probs = np.asarray(vocab_probs, dtype=np.float32)
    generation_probs = np.asarray(generation_probs, dtype=np.float32)
    agentwise_attn = np.asarray(agentwise_attn, dtype=np.float32)
    agent_attn = np.asarray(agent_attn, dtype=np.float32)
    article = np.asarray(article)

    if _prog is None:
        _prog = _build_program()

    in_maps = [
        _pack_core(
            vocab_probs[b], generation_probs[b], agat_b=agent_attn[b],
            attn_b=agentwise_attn[b], article_b=article[b],
        )
        for b in range(B)
    ]
    res = run_bass_kernel_spmd(_prog, in_maps, core_ids=list(range(B)))
    full = np.empty((B, T, EXT_V), np.float32)
    for b, r in enumerate(res.results):
        full[b] = _unpack(r)
    return full
